# revision 1
# baseline (speedup 1.0000x reference)
"""CBL (contrastive boundary) loss kernel for Trainium2, 8 NeuronCores.

Strategy (data-parallel over points, per spec sharding hint):
  - Shard the N=100000 points across 8 cores (12500 each, zero-padded to
    12544 = 128 partitions x 98 tiles).
  - Every core receives the full feature matrix, laid out as [N, 129]
    (x concatenated with target-as-float) so one 516-byte indirect-DMA row
    fetch brings both the neighbor features and the neighbor label.
  - Gathers use the [P, 1]-offset form (one row per partition per
    indirect DMA) — the only layout the SWDGE ucode implements; larger
    offset APs are simulator-only and read garbage offsets on hardware.
  - Per pair (i,k): dot(x_i, x_j) via DVE multiply + reduce,
    sum-of-squares of each gathered row via ACT square+accumulate, then
    dist = sqrt(max(2 - 2*cos, 0) + eps) with cos = dot * rsqrt(ss_i)
    * rsqrt(ss_j)  (algebraically identical to the reference's normalized
    L2 distance).
  - NCE contrast / masking / per-pair loss evaluated on-chip; each core
    emits partial (sum, count); host combines the 8 pairs (the scalar
    "all-reduce" of the sharding hint) and forms sum/max(cnt,1).
"""

import sys

if "/opt/trn_rl_repo" not in sys.path:
    sys.path.insert(0, "/opt/trn_rl_repo")

import numpy as np

N_TOTAL = 100000
C = 128
K = 7
NCORES = 8
P = 128
NSH = N_TOTAL // NCORES          # 12500 points per core
EPS = 1e-12
GB = 8                           # tiles per indirect-gather batch
FP16 = False                     # gather/dot in fp16 (halves gather bytes)


def xt_cols(fp16):
    # fp16 rows padded to an even element count (130*2B = 260B rows)
    return C + 2 if fp16 else C + 1


def build_nc(nsh=NSH, n_total=N_TOTAL, gb=GB, fp16=False):
    from concourse import bacc, bass  # noqa: F401
    import concourse.mybir as mybir
    from concourse.bass import IndirectOffsetOnAxis
    from concourse.tile import TileContext

    f32 = mybir.dt.float32
    i32 = mybir.dt.int32
    ft = mybir.dt.float16 if fp16 else f32
    XTC = xt_cols(fp16)
    Alu = mybir.AluOpType
    Act = mybir.ActivationFunctionType
    Ax = mybir.AxisListType

    T = (nsh + P - 1) // P       # tiles
    npad = T * P
    TK = T * K

    nc = bacc.Bacc(num_devices=NCORES)
    xs = nc.dram_tensor("xs", [npad, C], ft, kind="ExternalInput")
    xt = nc.dram_tensor("xt", [n_total, XTC], ft, kind="ExternalInput")
    idx = nc.dram_tensor("idx", [npad, K], i32, kind="ExternalInput")
    tgts = nc.dram_tensor("tgts", [npad], f32, kind="ExternalInput")
    part = nc.dram_tensor("part", [2], f32, kind="ExternalOutput")

    def seg(ap):
        # [P, T*K] flat view -> [P, T, K]
        return ap.rearrange("p (t k) -> p t k", k=K)

    with TileContext(nc) as tc:
        with (
            tc.tile_pool(name="cst", bufs=1) as cst,
            tc.tile_pool(name="nxp", bufs=8) as nxp,
            tc.tile_pool(name="psp", bufs=1, space="PSUM") as psp,
        ):
            # ---- resident loads (contiguous: point (p,t) = row p*T + t) ----
            xs_sb = cst.tile([P, T, C], ft)
            idx_sb = cst.tile([P, T, K], i32)
            tgts_sb = cst.tile([P, T], f32)
            nc.sync.dma_start(out=xs_sb[:], in_=xs[:, :].rearrange("(p t) c -> p t c", t=T))
            nc.sync.dma_start(out=idx_sb[:], in_=idx[:, :].rearrange("(p t) k -> p t k", t=T))
            nc.sync.dma_start(out=tgts_sb[:], in_=tgts[:].rearrange("(p t) -> p t", t=T))

            # ---- self sum-of-squares + r_i = sqrt(1/(ss+eps)) ----
            ss_sb = cst.tile([P, T], f32)
            sq_tr = cst.tile([P, C], ft)
            for t in range(T):
                nc.scalar.activation(out=sq_tr[:], in_=xs_sb[:, t, :], func=Act.Square,
                                     accum_out=ss_sb[:, t:t + 1])
            sse = cst.tile([P, T], f32)
            inv = cst.tile([P, T], f32)
            r_sb = cst.tile([P, T], f32)
            nc.vector.tensor_scalar_add(sse[:], ss_sb[:], EPS)
            nc.vector.reciprocal(inv[:], sse[:])
            nc.scalar.activation(out=r_sb[:], in_=inv[:], func=Act.Sqrt)

            # ---- gather neighbor rows, dots and neighbor norms ----
            dot_sb = cst.tile([P, TK], f32)
            ssn_sb = cst.tile([P, TK], f32)
            tgn_sb = cst.tile([P, TK], f32)
            ttr_tr = cst.tile([P, C], ft)
            sqn_tr = cst.tile([P, C], ft)
            # one row per partition per indirect DMA ([P,1] offsets — the form
            # the SWDGE ucode actually implements; multi-index offset APs are
            # sim-only)
            for t in range(T):
                for k in range(K):
                    j = t * K + k
                    nx = nxp.tile([P, XTC], ft, tag="nx")
                    nc.gpsimd.indirect_dma_start(
                        out=nx[:],
                        out_offset=None,
                        in_=xt[:, :],
                        in_offset=IndirectOffsetOnAxis(
                            ap=idx_sb[:, t, k:k + 1], axis=0),
                    )
                    nc.vector.tensor_copy(out=tgn_sb[:, j:j + 1],
                                          in_=nx[:, C:C + 1])
                    nc.scalar.activation(out=sqn_tr[:], in_=nx[:, 0:C],
                                         func=Act.Square,
                                         accum_out=ssn_sb[:, j:j + 1])
                    nc.vector.tensor_tensor(out=ttr_tr[:], in0=xs_sb[:, t, :],
                                            in1=nx[:, 0:C], op=Alu.mult)
                    nc.vector.tensor_reduce(out=dot_sb[:, j:j + 1], in_=ttr_tr[:],
                                            axis=Ax.X, op=Alu.add)

            # ---- phase B: per-pair loss on [P, T*K] ----
            ssne = cst.tile([P, TK], f32)
            invn = cst.tile([P, TK], f32)
            rn = cst.tile([P, TK], f32)
            nc.vector.tensor_scalar_add(ssne[:], ssn_sb[:], EPS)
            nc.vector.reciprocal(invn[:], ssne[:])
            nc.scalar.activation(out=rn[:], in_=invn[:], func=Act.Sqrt)

            t1 = cst.tile([P, TK], f32)
            cosv = cst.tile([P, TK], f32)
            nc.vector.tensor_tensor(out=t1[:], in0=dot_sb[:], in1=rn[:], op=Alu.mult)
            nc.vector.tensor_tensor(out=seg(cosv), in0=seg(t1),
                                    in1=r_sb[:, :, None].to_broadcast([P, T, K]),
                                    op=Alu.mult)
            # d2 = max(2 - 2*cos, 0); dist = sqrt(d2 + eps)
            d2 = cst.tile([P, TK], f32)
            nc.vector.tensor_scalar(d2[:], cosv[:], -2.0, 2.0, Alu.mult, Alu.add)
            d2c = cst.tile([P, TK], f32)
            nc.vector.tensor_scalar_max(d2c[:], d2[:], 0.0)
            eps_tile = cst.tile([P, 1], f32)
            nc.vector.memset(eps_tile[:], EPS)
            dist = cst.tile([P, TK], f32)
            nc.scalar.activation(out=dist[:], in_=d2c[:], func=Act.Sqrt,
                                 bias=eps_tile[:, 0:1])

            # M = max_k(-dist) = -min_k(dist); s = dist + M; e = exp(-s)
            M = cst.tile([P, T], f32)
            nc.vector.tensor_reduce(out=M[:], in_=seg(dist), axis=Ax.X, op=Alu.min,
                                    negate=True)
            s_t = cst.tile([P, TK], f32)
            nc.vector.tensor_tensor(out=seg(s_t), in0=seg(dist),
                                    in1=M[:, :, None].to_broadcast([P, T, K]),
                                    op=Alu.add)
            e_t = cst.tile([P, TK], f32)
            nc.scalar.activation(out=e_t[:], in_=s_t[:], func=Act.Exp, scale=-1.0)

            # posmask, npos, point_mask
            pos = cst.tile([P, TK], f32)
            nc.vector.tensor_tensor(out=seg(pos), in0=seg(tgn_sb),
                                    in1=tgts_sb[:, :, None].to_broadcast([P, T, K]),
                                    op=Alu.is_equal)
            npos = cst.tile([P, T], f32)
            nc.vector.tensor_reduce(out=npos[:], in_=seg(pos), axis=Ax.X, op=Alu.add)
            g1 = cst.tile([P, T], f32)
            g2 = cst.tile([P, T], f32)
            pm = cst.tile([P, T], f32)
            nc.vector.tensor_scalar(g1[:], npos[:], 0.5, None, Alu.is_gt)
            nc.vector.tensor_scalar(g2[:], npos[:], K - 0.5, None, Alu.is_lt)
            nc.vector.tensor_tensor(out=pm[:], in0=g1[:], in1=g2[:], op=Alu.mult)

            # neg = sum(e) - sum(e*pos); under = e + neg; L = ln(under)
            sall = cst.tile([P, T], f32)
            nc.vector.tensor_reduce(out=sall[:], in_=seg(e_t), axis=Ax.X, op=Alu.add)
            ep = cst.tile([P, TK], f32)
            nc.vector.tensor_tensor(out=ep[:], in0=e_t[:], in1=pos[:], op=Alu.mult)
            spos = cst.tile([P, T], f32)
            nc.vector.tensor_reduce(out=spos[:], in_=seg(ep), axis=Ax.X, op=Alu.add)
            neg = cst.tile([P, T], f32)
            nc.vector.tensor_tensor(out=neg[:], in0=sall[:], in1=spos[:],
                                    op=Alu.subtract)
            under = cst.tile([P, TK], f32)
            nc.vector.tensor_tensor(out=seg(under), in0=seg(e_t),
                                    in1=neg[:, :, None].to_broadcast([P, T, K]),
                                    op=Alu.add)
            Lt = cst.tile([P, TK], f32)
            nc.scalar.activation(out=Lt[:], in_=under[:], func=Act.Ln)

            # per_pair = L - log(e) = L + s ; contrib = per_pair * pos * pm
            pp = cst.tile([P, TK], f32)
            nc.vector.tensor_tensor(out=pp[:], in0=Lt[:], in1=s_t[:], op=Alu.add)
            sel = cst.tile([P, TK], f32)
            nc.vector.tensor_tensor(out=seg(sel), in0=seg(pos),
                                    in1=pm[:, :, None].to_broadcast([P, T, K]),
                                    op=Alu.mult)
            con = cst.tile([P, TK], f32)
            nc.vector.tensor_tensor(out=con[:], in0=pp[:], in1=sel[:], op=Alu.mult)

            # reduce to per-partition (sum, cnt), then across partitions via PE
            vals = cst.tile([P, 2], f32)
            nc.vector.tensor_reduce(out=vals[:, 0:1], in_=con[:], axis=Ax.X, op=Alu.add)
            nc.vector.tensor_reduce(out=vals[:, 1:2], in_=sel[:], axis=Ax.X, op=Alu.add)
            ones = cst.tile([P, 1], f32)
            nc.vector.memset(ones[:], 1.0)
            pst = psp.tile([2, 1], f32, space="PSUM")
            nc.tensor.matmul(out=pst[:], lhsT=vals[:], rhs=ones[:], start=True,
                             stop=True)
            res_sb = cst.tile([2, 1], f32)
            nc.vector.tensor_copy(out=res_sb[:], in_=pst[:])
            nc.sync.dma_start(out=part[:], in_=res_sb[:])
    nc.finalize()
    return nc


def make_in_maps(x, neighbor_idx, target, nsh=NSH, ncores=NCORES, fp16=False):
    """Shard + pad host-side (pure data marshaling)."""
    T = (nsh + P - 1) // P
    npad = T * P
    ftnp = np.float16 if fp16 else np.float32
    x = np.asarray(x, dtype=np.float32).astype(ftnp)
    idx_all = np.asarray(neighbor_idx).astype(np.int32)
    tgtf = np.asarray(target).astype(np.float32)
    n_total = x.shape[0]
    xt = np.zeros((n_total, xt_cols(fp16)), dtype=ftnp)
    xt[:, :C] = x
    xt[:, C] = tgtf
    in_maps = []
    for c in range(ncores):
        lo, hi = c * nsh, (c + 1) * nsh
        xs_pad = np.zeros((npad, C), dtype=ftnp)
        xs_pad[:nsh] = x[lo:hi]
        idx_pad = np.zeros((npad, K), dtype=np.int32)
        idx_pad[:nsh] = idx_all[lo:hi]
        tg_pad = np.full((npad,), -1.0, dtype=np.float32)
        tg_pad[:nsh] = tgtf[lo:hi]
        in_maps.append({"xs": xs_pad, "xt": xt, "idx": idx_pad, "tgts": tg_pad})
    return in_maps


def combine_parts(parts):
    parts = np.asarray(parts, dtype=np.float64)
    s = parts[:, 0].sum()
    cnt = parts[:, 1].sum()
    loss = s / max(cnt, 1.0) if cnt > 0 else 0.0
    return np.asarray(loss, dtype=np.float32)


def kernel(p, x, neighbor_idx, target):
    from concourse.bass_utils import run_bass_kernel_spmd

    in_maps = make_in_maps(x, neighbor_idx, target, fp16=FP16)
    nc = build_nc(fp16=FP16)
    res = run_bass_kernel_spmd(nc, in_maps, list(range(NCORES)))
    parts = [r["part"] for r in res.results]
    return combine_parts(parts)



# revision 12
# speedup vs baseline: 1.0183x; 1.0183x over previous
"""CBL (contrastive boundary) loss kernel for Trainium2, 8 NeuronCores.

Strategy (data-parallel over points, per spec sharding hint):
  - Shard the N=100000 points across 8 cores (12500 each, zero-padded to
    12544 = 128 partitions x 98 tiles).
  - Every core receives the full feature matrix, laid out as [N, 129]
    (x concatenated with target-as-float) so one 516-byte indirect-DMA row
    fetch brings both the neighbor features and the neighbor label.
  - Gathers use the [P, 1]-offset form (one row per partition per
    indirect DMA) — the only layout the SWDGE ucode implements; larger
    offset APs are simulator-only and read garbage offsets on hardware.
  - Per pair (i,k): dot(x_i, x_j) via DVE multiply + reduce,
    sum-of-squares of each gathered row via ACT square+accumulate, then
    dist = sqrt(max(2 - 2*cos, 0) + eps) with cos = dot * rsqrt(ss_i)
    * rsqrt(ss_j)  (algebraically identical to the reference's normalized
    L2 distance).
  - NCE contrast / masking / per-pair loss evaluated on-chip; each core
    emits partial (sum, count); host combines the 8 pairs (the scalar
    "all-reduce" of the sharding hint) and forms sum/max(cnt,1).
"""

import sys

if "/opt/trn_rl_repo" not in sys.path:
    sys.path.insert(0, "/opt/trn_rl_repo")

import numpy as np

N_TOTAL = 100000
C = 128
K = 7
NCORES = 8
P = 128
NSH = N_TOTAL // NCORES          # 12500 points per core
EPS = 1e-12
GB = 8                           # tiles per indirect-gather batch
FP16 = True                     # gather/dot in fp16 (halves gather bytes)


def xt_cols(fp16):
    # fp16 rows padded to an even element count (130*2B = 260B rows)
    return C + 2 if fp16 else C + 1


def build_nc(nsh=NSH, n_total=N_TOTAL, gb=GB, fp16=False):
    from concourse import bacc, bass  # noqa: F401
    import concourse.mybir as mybir
    from concourse.bass import IndirectOffsetOnAxis
    from concourse.tile import TileContext

    f32 = mybir.dt.float32
    i32 = mybir.dt.int32
    ft = mybir.dt.float16 if fp16 else f32
    XTC = xt_cols(fp16)
    Alu = mybir.AluOpType
    Act = mybir.ActivationFunctionType
    Ax = mybir.AxisListType

    T = (nsh + P - 1) // P       # tiles
    npad = T * P
    TK = T * K

    nc = bacc.Bacc(num_devices=NCORES)
    xs = nc.dram_tensor("xs", [npad, C], ft, kind="ExternalInput")
    xt = nc.dram_tensor("xt", [n_total, XTC], ft, kind="ExternalInput")
    idx = nc.dram_tensor("idx", [npad, K], i32, kind="ExternalInput")
    tgts = nc.dram_tensor("tgts", [npad], f32, kind="ExternalInput")
    part = nc.dram_tensor("part", [2], f32, kind="ExternalOutput")

    def seg(ap):
        # [P, T*K] flat view -> [P, T, K]
        return ap.rearrange("p (t k) -> p t k", k=K)

    with TileContext(nc) as tc:
        with (
            tc.tile_pool(name="cst", bufs=1) as cst,
            tc.tile_pool(name="nxp", bufs=8) as nxp,
            tc.tile_pool(name="psp", bufs=1, space="PSUM") as psp,
        ):
            # ---- resident loads (contiguous: point (p,t) = row p*T + t) ----
            xs_sb = cst.tile([P, T, C], ft)
            idx_sb = cst.tile([P, T, K], i32)
            tgts_sb = cst.tile([P, T], f32)
            nc.sync.dma_start(out=xs_sb[:], in_=xs[:, :].rearrange("(p t) c -> p t c", t=T))
            nc.sync.dma_start(out=idx_sb[:], in_=idx[:, :].rearrange("(p t) k -> p t k", t=T))
            nc.sync.dma_start(out=tgts_sb[:], in_=tgts[:].rearrange("(p t) -> p t", t=T))

            # ---- self sum-of-squares + r_i = sqrt(1/(ss+eps)) ----
            ss_sb = cst.tile([P, T], f32)
            sq_tr = cst.tile([P, C], ft)
            for t in range(T):
                nc.scalar.activation(out=sq_tr[:], in_=xs_sb[:, t, :], func=Act.Square,
                                     accum_out=ss_sb[:, t:t + 1])
            sse = cst.tile([P, T], f32)
            inv = cst.tile([P, T], f32)
            r_sb = cst.tile([P, T], f32)
            nc.vector.tensor_scalar_add(sse[:], ss_sb[:], EPS)
            nc.vector.reciprocal(inv[:], sse[:])
            nc.scalar.activation(out=r_sb[:], in_=inv[:], func=Act.Sqrt)

            # ---- gather neighbor rows, dots and neighbor norms ----
            dot_sb = cst.tile([P, TK], f32)
            ssn_sb = cst.tile([P, TK], f32)
            tgn_sb = cst.tile([P, TK], f32)
            ttr_tr = cst.tile([P, C], ft)
            sqn_tr = cst.tile([P, C], ft)
            # one row per partition per indirect DMA ([P,1] offsets — the form
            # the SWDGE ucode actually implements; multi-index offset APs are
            # sim-only)
            for t in range(T):
                for k in range(K):
                    j = t * K + k
                    nx = nxp.tile([P, XTC], ft, tag="nx")
                    nc.gpsimd.indirect_dma_start(
                        out=nx[:],
                        out_offset=None,
                        in_=xt[:, :],
                        in_offset=IndirectOffsetOnAxis(
                            ap=idx_sb[:, t, k:k + 1], axis=0),
                    )
                    nc.vector.tensor_copy(out=tgn_sb[:, j:j + 1],
                                          in_=nx[:, C:C + 1])
                    nc.scalar.activation(out=sqn_tr[:], in_=nx[:, 0:C],
                                         func=Act.Square,
                                         accum_out=ssn_sb[:, j:j + 1])
                    nc.vector.tensor_tensor(out=ttr_tr[:], in0=xs_sb[:, t, :],
                                            in1=nx[:, 0:C], op=Alu.mult)
                    nc.vector.tensor_reduce(out=dot_sb[:, j:j + 1], in_=ttr_tr[:],
                                            axis=Ax.X, op=Alu.add)

            # ---- phase B: per-pair loss on [P, T*K] ----
            ssne = cst.tile([P, TK], f32)
            invn = cst.tile([P, TK], f32)
            rn = cst.tile([P, TK], f32)
            nc.vector.tensor_scalar_add(ssne[:], ssn_sb[:], EPS)
            nc.vector.reciprocal(invn[:], ssne[:])
            nc.scalar.activation(out=rn[:], in_=invn[:], func=Act.Sqrt)

            t1 = cst.tile([P, TK], f32)
            cosv = cst.tile([P, TK], f32)
            nc.vector.tensor_tensor(out=t1[:], in0=dot_sb[:], in1=rn[:], op=Alu.mult)
            nc.vector.tensor_tensor(out=seg(cosv), in0=seg(t1),
                                    in1=r_sb[:, :, None].to_broadcast([P, T, K]),
                                    op=Alu.mult)
            # d2 = max(2 - 2*cos, 0); dist = sqrt(d2 + eps)
            d2 = cst.tile([P, TK], f32)
            nc.vector.tensor_scalar(d2[:], cosv[:], -2.0, 2.0, Alu.mult, Alu.add)
            d2c = cst.tile([P, TK], f32)
            nc.vector.tensor_scalar_max(d2c[:], d2[:], 0.0)
            eps_tile = cst.tile([P, 1], f32)
            nc.vector.memset(eps_tile[:], EPS)
            dist = cst.tile([P, TK], f32)
            nc.scalar.activation(out=dist[:], in_=d2c[:], func=Act.Sqrt,
                                 bias=eps_tile[:, 0:1])

            # M = max_k(-dist) = -min_k(dist); s = dist + M; e = exp(-s)
            M = cst.tile([P, T], f32)
            nc.vector.tensor_reduce(out=M[:], in_=seg(dist), axis=Ax.X, op=Alu.min,
                                    negate=True)
            s_t = cst.tile([P, TK], f32)
            nc.vector.tensor_tensor(out=seg(s_t), in0=seg(dist),
                                    in1=M[:, :, None].to_broadcast([P, T, K]),
                                    op=Alu.add)
            e_t = cst.tile([P, TK], f32)
            nc.scalar.activation(out=e_t[:], in_=s_t[:], func=Act.Exp, scale=-1.0)

            # posmask, npos, point_mask
            pos = cst.tile([P, TK], f32)
            nc.vector.tensor_tensor(out=seg(pos), in0=seg(tgn_sb),
                                    in1=tgts_sb[:, :, None].to_broadcast([P, T, K]),
                                    op=Alu.is_equal)
            npos = cst.tile([P, T], f32)
            nc.vector.tensor_reduce(out=npos[:], in_=seg(pos), axis=Ax.X, op=Alu.add)
            g1 = cst.tile([P, T], f32)
            g2 = cst.tile([P, T], f32)
            pm = cst.tile([P, T], f32)
            nc.vector.tensor_scalar(g1[:], npos[:], 0.5, None, Alu.is_gt)
            nc.vector.tensor_scalar(g2[:], npos[:], K - 0.5, None, Alu.is_lt)
            nc.vector.tensor_tensor(out=pm[:], in0=g1[:], in1=g2[:], op=Alu.mult)

            # neg = sum(e) - sum(e*pos); under = e + neg; L = ln(under)
            sall = cst.tile([P, T], f32)
            nc.vector.tensor_reduce(out=sall[:], in_=seg(e_t), axis=Ax.X, op=Alu.add)
            ep = cst.tile([P, TK], f32)
            nc.vector.tensor_tensor(out=ep[:], in0=e_t[:], in1=pos[:], op=Alu.mult)
            spos = cst.tile([P, T], f32)
            nc.vector.tensor_reduce(out=spos[:], in_=seg(ep), axis=Ax.X, op=Alu.add)
            neg = cst.tile([P, T], f32)
            nc.vector.tensor_tensor(out=neg[:], in0=sall[:], in1=spos[:],
                                    op=Alu.subtract)
            under = cst.tile([P, TK], f32)
            nc.vector.tensor_tensor(out=seg(under), in0=seg(e_t),
                                    in1=neg[:, :, None].to_broadcast([P, T, K]),
                                    op=Alu.add)
            Lt = cst.tile([P, TK], f32)
            nc.scalar.activation(out=Lt[:], in_=under[:], func=Act.Ln)

            # per_pair = L - log(e) = L + s ; contrib = per_pair * pos * pm
            pp = cst.tile([P, TK], f32)
            nc.vector.tensor_tensor(out=pp[:], in0=Lt[:], in1=s_t[:], op=Alu.add)
            sel = cst.tile([P, TK], f32)
            nc.vector.tensor_tensor(out=seg(sel), in0=seg(pos),
                                    in1=pm[:, :, None].to_broadcast([P, T, K]),
                                    op=Alu.mult)
            con = cst.tile([P, TK], f32)
            nc.vector.tensor_tensor(out=con[:], in0=pp[:], in1=sel[:], op=Alu.mult)

            # reduce to per-partition (sum, cnt), then across partitions via PE
            vals = cst.tile([P, 2], f32)
            nc.vector.tensor_reduce(out=vals[:, 0:1], in_=con[:], axis=Ax.X, op=Alu.add)
            nc.vector.tensor_reduce(out=vals[:, 1:2], in_=sel[:], axis=Ax.X, op=Alu.add)
            ones = cst.tile([P, 1], f32)
            nc.vector.memset(ones[:], 1.0)
            pst = psp.tile([2, 1], f32, space="PSUM")
            nc.tensor.matmul(out=pst[:], lhsT=vals[:], rhs=ones[:], start=True,
                             stop=True)
            res_sb = cst.tile([2, 1], f32)
            nc.vector.tensor_copy(out=res_sb[:], in_=pst[:])
            nc.sync.dma_start(out=part[:], in_=res_sb[:])
    nc.finalize()
    return nc


def make_in_maps(x, neighbor_idx, target, nsh=NSH, ncores=NCORES, fp16=False):
    """Shard + pad host-side (pure data marshaling)."""
    T = (nsh + P - 1) // P
    npad = T * P
    ftnp = np.float16 if fp16 else np.float32
    x = np.asarray(x, dtype=np.float32).astype(ftnp)
    idx_all = np.asarray(neighbor_idx).astype(np.int32)
    tgtf = np.asarray(target).astype(np.float32)
    n_total = x.shape[0]
    xt = np.zeros((n_total, xt_cols(fp16)), dtype=ftnp)
    xt[:, :C] = x
    xt[:, C] = tgtf
    in_maps = []
    for c in range(ncores):
        lo, hi = c * nsh, (c + 1) * nsh
        xs_pad = np.zeros((npad, C), dtype=ftnp)
        xs_pad[:nsh] = x[lo:hi]
        idx_pad = np.zeros((npad, K), dtype=np.int32)
        idx_pad[:nsh] = idx_all[lo:hi]
        tg_pad = np.full((npad,), -1.0, dtype=np.float32)
        tg_pad[:nsh] = tgtf[lo:hi]
        in_maps.append({"xs": xs_pad, "xt": xt, "idx": idx_pad, "tgts": tg_pad})
    return in_maps


def combine_parts(parts):
    parts = np.asarray(parts, dtype=np.float64)
    s = parts[:, 0].sum()
    cnt = parts[:, 1].sum()
    loss = s / max(cnt, 1.0) if cnt > 0 else 0.0
    return np.asarray(loss, dtype=np.float32)


def kernel(p, x, neighbor_idx, target):
    from concourse.bass_utils import run_bass_kernel_spmd

    in_maps = make_in_maps(x, neighbor_idx, target, fp16=FP16)
    nc = build_nc(fp16=FP16)
    res = run_bass_kernel_spmd(nc, in_maps, list(range(NCORES)))
    parts = [r["part"] for r in res.results]
    return combine_parts(parts)

